# revision 8
# baseline (speedup 1.0000x reference)
"""Trainium2 Bass kernel for nn_LocalizationLoss (optimized).

Loss (see reference):
  p = out[:,:,0]; t = tgt[:,:,0] in {0,1}
  bce  = -mean(t*ln(p) + (1-t)*ln(1-p))
  trick= out * t[...,None]
  CE over slot axis (dim 1) of trick[:,:,4:7] with targets tgt[:,:,4]
  Lx   = mean((trick_x - tx)^2), Ly likewise
  Lwh  = mean((t*sqrt(ow) - sqrt(tw))^2)
  loss = 5*(Lx+Ly+2*Lwh) + bce + 0.5*(1-bce) + 3*ce

Per core (batch-sharded), per-partition partial sums (bf16 compute,
f32 accumulation via per-instruction accum_out):
  PE (ones-matmul, PSUM accum): sum tw + sum t*ow  (only their sum is
            needed: Swh = sum(t*ow) + sum(tw) - TS2), and the CE select
            SEL = sum l1 + sum H0*(l0-l1) + sum H2*(l2-l1) with
            H0 = relu(1-tgt), H2 = relu(tgt-1) (ACT) since H1 = 1-H0-H2
  ACT cols: BCE  = sum ln((p+t-1)^2 + 1e-6)          (= 2*sum ln|p+t-1|)
            SQXY = sum (t*ox-tx)^2 + (t*oy-ty)^2
            TS2  = sum 2*sqrt(t*ow*tw + 1e-12)       (= sum t*2*sqrt(ow*tw))
            LSE  = sum_j ln sum_i exp(t_i*o_i[4+j])
Host:  Swh = MWA + TW - TS2   [(t*sqrt(ow)-sqrt(tw))^2 == t*ow + tw
                               - 2*t*sqrt(ow*tw) for t in {0,1}]
       ce*3B = LSE - (SEL0+SEL1+SEL2);  bce*3B = -0.5*BCE
  loss = 0.5 + (5*SQXY + 10*Swh - 0.25*BCE + 3*(LSE-SEL))/ (3B)

Structure: 4 equal chunks (rows/partition), SWDGE f32->bf16 cast DMA
loads issued 2 chunks ahead on the gpsimd queue; masked logits stored
in (i, g, j) order so the CE class-select reads targets without
staging; all reductions fused into producer instructions (accum_out).
"""

import numpy as np

import concourse.bass as bass
import concourse.bacc as bacc
import concourse.mybir as mybir
from concourse.tile import TileContext
from concourse.bass_utils import run_bass_kernel_spmd

# Force the ACT table pass to use only natural_log_exp_and_others (it holds
# every func this kernel needs: ln/exp/square/copy). The default greedy
# per-func set choice thrashes between sets, costing a ~1.3us
# ACT_TABLE_LOAD each time. Blank the other sets, keep dict order so
# act_func_set_id indices stay aligned with act_info.json.
import concourse.hw_specs as _hw_specs
if not hasattr(_hw_specs, "_orig_get_activation_tables"):
    _hw_specs._orig_get_activation_tables = _hw_specs.get_activation_tables

    def _only_ln_exp_tables(module_arch):
        tabs = _hw_specs._orig_get_activation_tables(module_arch)
        return {
            name: (funcs if name == "natural_log_exp_and_others" else set())
            for name, funcs in tabs.items()
        }

    _hw_specs.get_activation_tables = _only_ln_exp_tables
    import concourse.bacc as _bacc_mod
    if hasattr(_bacc_mod, "get_activation_tables"):
        _bacc_mod.get_activation_tables = _only_ln_exp_tables

F32 = mybir.dt.float32
BF16 = mybir.dt.bfloat16
ALU = mybir.AluOpType
ACT = mybir.ActivationFunctionType
LN2 = 0.6931471805599453

P = 128
N_CORES = 8
B_FULL = 1_048_576

(A_BCE, A_SQXY, A_TS2, A_LSE) = range(4)

CHUNKS_FULL = (384, 960, 864, 864)   # rpp = 3072; small first chunk
# shortens the pipeline-fill (first-load latency) with no measured
# steady-state cost vs equal 768x4 chunks


def build_kernel(nb: int, chunks) -> bass.Bass:
    """Build the per-core Bass program for nb batch elements (ROWS=nb*3)."""
    rows = nb * 3
    assert rows % P == 0
    rpp = rows // P
    chunks = list(chunks)
    assert sum(chunks) == rpp and all(r % 3 == 0 for r in chunks)
    n_chunks = len(chunks)

    nc = bacc.Bacc()
    # consts on DVE: keeps the Pool queue free so the first DMA
    # descriptor-gen starts immediately after the barrier
    for val in (-1.0, 1e-6, 1e-12, LN2):
        ctile = nc.alloc_sbuf_tensor(f"const-f32-{val}", [128, 1], F32)
        nc.vector.memset(ctile.ap(), val)
        nc.const_aps.aps[(F32, val)] = ctile.ap()
    ones = nc.alloc_sbuf_tensor("ones-bf16", [128, 1], BF16)
    nc.vector.memset(ones.ap(), 1.0)
    nc.all_engine_barrier()

    out_hbm = nc.declare_dram_parameter("output", [rows * 7], F32, isOutput=False)
    tgt_hbm = nc.declare_dram_parameter("target", [rows * 5], F32, isOutput=False)
    resa_hbm = nc.declare_dram_parameter("res_act", [P, 4 * n_chunks], F32, isOutput=True)
    respe_hbm = nc.declare_dram_parameter("res_pe", [1, 2560], F32, isOutput=True)

    out_v = out_hbm[:].rearrange("(p n) -> p n", p=P)
    tgt_v = tgt_hbm[:].rearrange("(p n) -> p n", p=P)

    sched = []
    row0 = 0
    for c, R in enumerate(chunks):
        sched.append((c, row0, R))
        row0 += R

    n_mm = 5 * sum(len(range(0, R, 512)) for R in chunks)
    with TileContext(nc) as tc:
        with (
            tc.tile_pool(name="io", bufs=3) as io_pool,
            tc.tile_pool(name="mid", bufs=2) as mid_pool,
            tc.tile_pool(name="accp", bufs=1) as acc_pool,
            tc.tile_pool(name="psum", bufs=1, space=bass.MemorySpace.PSUM) as ps_pool,
        ):
            cols_a = acc_pool.tile([P, 4 * n_chunks], F32)
            pacc = ps_pool.tile([1, 5, 512], F32)
            pe_out = acc_pool.tile([1, 2560], F32)
            nc.vector.memset(pacc[0:1, :, :], 0.0)
            mm_idx = 0

            def issue_load(row0, R):
                ot = io_pool.tile([P, R * 7], BF16, tag="ot")
                tt = io_pool.tile([P, R * 5], BF16, tag="tt")
                # gpsimd (SWDGE) DMA casts f32 DRAM -> bf16 SBUF for free
                nc.gpsimd.dma_start(out=ot[:, :], in_=out_v[:, row0 * 7:(row0 + R) * 7])
                nc.gpsimd.dma_start(out=tt[:, :], in_=tgt_v[:, row0 * 5:(row0 + R) * 5])
                return ot, tt

            PF = 2   # chunks of DMA prefetch ahead of compute
            pending = [issue_load(r0, R) for (_, r0, R) in sched[:PF]]
            for si, (c, row0, R) in enumerate(sched):
                ot, tt = pending.pop(0)
                if si + PF < len(sched):
                    pending.append(issue_load(sched[si + PF][1], sched[si + PF][2]))
                ab = c * 4
                G = R // 3

                o3 = ot[:, :].rearrange("p (r c) -> p r c", c=7)
                t5 = tt[:, :].rearrange("p (r c) -> p r c", c=5)
                p_ch = o3[:, :, 0]
                ow = o3[:, :, 3]
                t_ch = t5[:, :, 0]
                tw = t5[:, :, 3]

                Mxyw = mid_pool.tile([P, R * 3], BF16, tag="Mxyw")
                Mlog = mid_pool.tile([P, R * 3], BF16, tag="Mlog")
                E = mid_pool.tile([P, R * 3], BF16, tag="E")
                S = mid_pool.tile([P, R], BF16, tag="S")
                exy = mid_pool.tile([P, R * 2], BF16, tag="exy")
                mq = mid_pool.tile([P, R], BF16, tag="mq")
                qs = mid_pool.tile([P, R], BF16, tag="qs")
                lnm = mid_pool.tile([P, R], F32, tag="lnm")
                junk = mid_pool.tile([P, R], BF16, tag="junk")
                junkb = mid_pool.tile([P, R], BF16, tag="junkb")

                # --- DVE: masked planes x, y, w in one op ---
                o_xyw = ot[:, :].rearrange("p (r c) -> p c r", c=7)[:, 1:4, :]
                t_b3 = t5[:, :, 0:1].broadcast_to([P, R, 3]).rearrange("p r c -> p c r")
                nc.vector.tensor_tensor(
                    Mxyw[:, :].rearrange("p (c r) -> p c r", c=3), o_xyw, t_b3, ALU.mult)
                Mxy = Mxyw[:, 0:2 * R]
                Mw = Mxyw[:, 2 * R:3 * R]
                # masked logits, (i, g, j) plane order
                Mlog_igj = Mlog[:, :].rearrange("p (i g j) -> p i g j", i=3, j=3)
                o_lg = ot[:, :].rearrange("p (g i c) -> p i g c", i=3, c=7)[:, :, :, 4:7]
                t_bl = (
                    tt[:, :].rearrange("p (g i c) -> p i g c", i=3, c=5)[:, :, :, 0:1]
                    .broadcast_to([P, 3, G, 3])
                )
                nc.vector.tensor_tensor(Mlog_igj, o_lg, t_bl, ALU.mult)
                nc.vector.tensor_tensor(qs[:, :], p_ch, t_ch, ALU.add)

                # --- PE: sum(tw) + sum(t*ow) accumulated into shared PSUM ---
                for base in range(0, R, 512):
                    w = min(512, R - base)
                    nc.tensor.matmul(
                        pacc[0:1, 0, 0:w], ones.ap()[:, 0:1],
                        t5[:, base:base + w, 3],
                        start=False, stop=(mm_idx == n_mm - 1),
                        skip_group_check=True)
                    mm_idx += 1
                    nc.tensor.matmul(
                        pacc[0:1, 1, 0:w], ones.ap()[:, 0:1],
                        Mw[:, base:base + w],
                        start=False, stop=(mm_idx == n_mm - 1),
                        skip_group_check=True)
                    mm_idx += 1

                # --- BCE: sum ln((p+t-1)^2 + 1e-6); host scales by 0.5 ---
                nc.scalar.activation(qs[:, :], qs[:, :], ACT.Square, bias=-1.0)
                nc.scalar.activation(
                    junk[:, :], qs[:, :], ACT.Ln, bias=1e-6,
                    accum_out=cols_a[:, ab + A_BCE:ab + A_BCE + 1])
                # --- x/y MSE ---
                t_xy = t5[:, :, 1:3].rearrange("p r c -> p c r")
                nc.vector.tensor_tensor(
                    exy[:, :].rearrange("p (c r) -> p c r", c=2),
                    Mxy.rearrange("p (c r) -> p c r", c=2), t_xy, ALU.subtract)
                nc.scalar.activation(
                    exy[:, :], exy[:, :], ACT.Square,
                    accum_out=cols_a[:, ab + A_SQXY:ab + A_SQXY + 1])
                # --- wh: sum 2*sqrt(t*ow*tw + eps) via exp(0.5*ln(.)+ln2) ---
                nc.vector.tensor_tensor(mq[:, :], Mw, tw, ALU.mult)
                nc.scalar.activation(lnm[:, :], mq[:, :], ACT.Ln, bias=1e-12)
                nc.scalar.activation(
                    junkb[:, :], lnm[:, :], ACT.Exp, bias=LN2, scale=0.5,
                    accum_out=cols_a[:, ab + A_TS2:ab + A_TS2 + 1])
                # --- CE: lse over slots; select via (tgt==i) masks ---
                nc.scalar.activation(E[:, :], Mlog[:, :], ACT.Exp)
                E_pl = E[:, :].rearrange("p (i r) -> p i r", i=3)
                nc.vector.tensor_tensor(S[:, :], E_pl[:, 0], E_pl[:, 1], ALU.add)
                nc.vector.tensor_tensor(S[:, :], S[:, :], E_pl[:, 2], ALU.add)
                nc.scalar.activation(
                    S[:, :], S[:, :], ACT.Ln,
                    accum_out=cols_a[:, ab + A_LSE:ab + A_LSE + 1])
                # CE select via H1 = 1 - H0 - H2 (tgt in {0,1,2}):
                #   SEL = sum(l1) + sum(H0*(l0-l1)) + sum(H2*(l2-l1))
                tgt_gj = tt[:, :].rearrange("p (g i c) -> p g i c", i=3, c=5)[:, :, :, 4]
                h02 = mid_pool.tile([P, R * 2], BF16, tag="h02")
                d02 = mid_pool.tile([P, R * 2], BF16, tag="d02")
                nc.scalar.activation(
                    h02[:, 0:R].rearrange("p (g j) -> p g j", j=3), tgt_gj,
                    ACT.Relu, bias=1.0, scale=-1.0)
                nc.scalar.activation(
                    h02[:, R:2 * R].rearrange("p (g j) -> p g j", j=3), tgt_gj,
                    ACT.Relu, bias=-1.0, scale=1.0)
                l1 = Mlog[:, R:2 * R]
                # (l0, l2) planes as one 2-run view; l1 broadcast across runs
                l02 = Mlog[:, :].rearrange(
                    "p (i r) -> p i r", i=3)[:, 0:3:2, :]
                l1_b = Mlog[:, :].rearrange(
                    "p (i r) -> p i r", i=3)[:, 1:2, :].broadcast_to([P, 2, R])
                d02_v = d02[:, :].rearrange("p (a r) -> p a r", a=2)
                nc.vector.tensor_tensor(d02_v, l02, l1_b, ALU.subtract)
                nc.vector.tensor_tensor(d02[:, :], d02[:, :], h02[:, :], ALU.mult)
                for base in range(0, R, 512):
                    w = min(512, R - base)
                    for q, blk in ((2, l1[:, base:base + w]),
                                   (3, d02[:, base:base + w]),
                                   (4, d02[:, R + base:R + base + w])):
                        nc.tensor.matmul(
                            pacc[0:1, q, 0:w], ones.ap()[:, 0:1], blk,
                            start=False, stop=(mm_idx == n_mm - 1),
                            skip_group_check=True)
                        mm_idx += 1

            # PSUM -> SBUF on ACT: off the DVE drain tail
            nc.scalar.activation(
                pe_out[0:1, :], pacc[0:1, :, :].rearrange("p a b -> p (a b)"),
                ACT.Copy)
            nc.sync.dma_start(out=resa_hbm[:, :], in_=cols_a[:, :])
            nc.sync.dma_start(out=respe_hbm[:, :], in_=pe_out[0:1, :])

    nc.compile()
    return nc


def combine_results(res_list, n_chunks: int, b_total: int) -> np.float32:
    """Host-side combine of per-core (res_act, res_pe) partial sums."""
    acca = np.zeros(4, dtype=np.float64)
    s_mwtw = 0.0
    s_sel = 0.0
    for ra, rp in res_list:
        acca += np.asarray(ra).astype(np.float64).reshape(P, n_chunks, 4).sum(axis=(0, 1))
        rpv = np.asarray(rp).astype(np.float64).reshape(5, 512)
        s_mwtw += rpv[0:2].sum()
        s_sel += rpv[2:5].sum()
    s_bce = 0.5 * acca[A_BCE]
    s_sqxy = acca[A_SQXY]
    s_wh = s_mwtw - acca[A_TS2]
    s_ce = acca[A_LSE] - s_sel
    denom = 3.0 * b_total
    loss = 0.5 + (5.0 * s_sqxy + 10.0 * s_wh - 0.5 * s_bce + 3.0 * s_ce) / denom
    return np.float32(loss)


_CACHED = {}


def _chunks_for(nb: int):
    rpp = nb * 3 // P
    if rpp == 3072:
        return CHUNKS_FULL
    for n in (4, 2, 1):
        if rpp % n == 0 and (rpp // n) % 3 == 0:
            return (rpp // n,) * n
    return (rpp,)


def _get_nc(nb: int):
    chunks = _chunks_for(nb)
    key = (nb, chunks)
    if key not in _CACHED:
        _CACHED[key] = (build_kernel(nb, chunks), len(chunks))
    return _CACHED[key]


def run_on_cores(output: np.ndarray, target: np.ndarray, trace: bool = False):
    """Shard along batch, run on 8 cores, return (res_list, n_chunks, results)."""
    b = output.shape[0]
    nb = b // N_CORES
    nc, n_chunks = _get_nc(nb)
    in_maps = []
    for k in range(N_CORES):
        o = np.ascontiguousarray(output[k * nb:(k + 1) * nb]).reshape(-1)
        t = np.ascontiguousarray(target[k * nb:(k + 1) * nb]).reshape(-1)
        in_maps.append({"output": o, "target": t})
    results = run_bass_kernel_spmd(
        nc, in_maps, core_ids=list(range(N_CORES)), trace=trace
    )
    res_list = [(r["res_act"], r["res_pe"]) for r in results.results]
    return res_list, n_chunks, results


def kernel(output: np.ndarray, target: np.ndarray) -> np.ndarray:
    output = np.asarray(output, dtype=np.float32)
    target = np.asarray(target, dtype=np.float32)
    b = output.shape[0]
    res_list, n_chunks, _ = run_on_cores(output, target)
    return combine_results(res_list, n_chunks=n_chunks, b_total=b)
